# revision 1
# baseline (speedup 1.0000x reference)
"""Trainium2 Bass kernel for nn_ClusterMemory (scatter_memory).

Strategy
--------
Column-shard ("tensor parallel") the three memory banks along num_samples:
core c owns bank columns [c*2048, (c+1)*2048).  Every core receives the full
(l2-normalized, transposed, bf16) student batch and computes its [1024, 2048]
block of the three similarity matrices C_b = x_b @ F_b^T on the PE in bf16.

Loss decomposition (all cross-core combination is a sum of per-core
per-row partial reductions, done on host):

  CE(out_b)    = mean_i [ log(sum_j exp(C/T)) - C[i,t_i]/T ]
                 -> device: row-sums of exp(C/T) via ACT Exp+accum.
                 -> C[i,t_i] = <x_i, f_{t_i}> via per-core row-slice dot with
                    host-gathered target rows (DVE scalar_tensor_tensor+accum).
  MSE ld_b     = sum_d mean_i (x - t)^2  -> device row partials (DVE sub +
                 ACT Square+accum on the core's 128-row slice).
  CE(soft_b)   = mean_i [ log(sum_j exp(s_ij)) - s[i,t_i] ],
                 s = softmax_j(dist).  dist in [0,2] => s_ij <= ~1e-4, so
                 sum_j exp(s_ij) = N + sum_j s_ij + O(sum s^2) = N + 1 + ~3e-5
                 (error ~2e-9 in the log).  Only Zd_i = sum_j exp(dist_ij)
                 is data-dependent, and it only enters through
                 s_t = exp(d_t)/Zd ~ 6e-5, so Zd tolerates ~1e-3 rel error:
                 exp(sqrt(2-2c)) is replaced by its quadratic fit
                 a2*(c+beta)^2 + c0 on the achievable domain of c, evaluated
                 as a single ACT Square (bias=beta) with accum, with the
                 affine applied on host.  This keeps the Scalar engine in the
                 "exp" activation-table set for the whole kernel (no ~2.7us
                 table switches).
"""

import numpy as np
import ml_dtypes

import bass_rust
import concourse.bass as bass
import concourse.tile as tile
from concourse import mybir
from concourse.bass_utils import run_bass_kernel_spmd

B, D, N = 1024, 2048, 16384
TEMP, LAMBDA2, MU = 0.05, 0.5, 1.0
NCORES = 8
JSH = N // NCORES          # 2048 bank columns per core
RSH = B // NCORES          # 128-row slice per core for MSE / target dots
KT = D // 128              # 16 contraction tiles
NIT = B // 128             # 8 row tiles
NJC = 2                    # j chunks per core (1024 wide each)
JCW = JSH // NJC           # 1024
NSLOT = 3 * NJC * NIT      # 48 accumulation slots

BF16 = ml_dtypes.bfloat16

# quadratic fit of f(c) = exp(sqrt(2 - 2c)) on the reachable cosine domain
_c = np.linspace(-0.35, 0.35, 4001)
_a2, _a1, _a0 = np.polyfit(_c, np.exp(np.sqrt(2.0 - 2.0 * _c)), 2)
QBETA = float(_a1 / (2.0 * _a2))          # Square bias
QA2 = float(_a2)                          # host-side scale
QC0 = float(_a0 - _a1 * _a1 / (4.0 * _a2))  # host-side offset

_NC_CACHE = {}
TRACE = False
TRACE_KWARGS = {}
LAST_RESULTS = None
LEGALIZE = True  # CoreSim needs the pre-legalized program; hardware needs it


def _legalize_sync_waits(nc):
    """The walrus build in this container encodes at most one sync wait per
    instruction; hoist extra waits into standalone EventSemaphore sequencer
    instructions on the same engine immediately before the instruction
    (identical semantics: the sequencer blocks before issuing)."""
    f = nc.m.functions[0]
    for blk in f.blocks:
        out = []
        for ins in blk.instructions:
            si = ins.sync_info
            if si is not None:
                waits = list(si.on_wait)
                ups = list(si.on_update or [])
                assert len(ups) <= 1, ins.concise()
                if len(waits) > 1:
                    for w in waits[:-1]:
                        ev = mybir.InstEventSemaphore(
                            name=f"lgw-{nc.next_id()}", ins=[], outs=[])
                        ev.engine = ins.engine
                        ev.sync_info = bass_rust.SyncInfo(on_wait=[w],
                                                          on_update=[])
                        out.append(ev)
                    ins.sync_info = bass_rust.SyncInfo(on_wait=[waits[-1]],
                                                      on_update=ups)
            out.append(ins)
        blk.instructions = out


def _build_nc(reps=1, skip_act=False, skip_mm=False):
    f32 = mybir.dt.float32
    bf16 = mybir.dt.bfloat16
    nc = bass.Bass("TRN2", target_bir_lowering=False, debug=False,
                   num_devices=NCORES)

    xt_d = [nc.dram_tensor(f"xt{b}", [D, B], bf16, kind="ExternalInput")
            for b in range(3)]
    ft_d = [nc.dram_tensor(f"ft{b}", [D, JSH], bf16, kind="ExternalInput")
            for b in range(3)]
    xs_d = [nc.dram_tensor(f"xs{b}", [RSH, D], bf16, kind="ExternalInput")
            for b in range(3)]
    ts_d = [nc.dram_tensor(f"tn{b}", [RSH, D], bf16, kind="ExternalInput")
            for b in range(3)]
    g_d = [nc.dram_tensor(f"g{b}", [RSH, D], bf16, kind="ExternalInput")
           for b in range(3)]
    zout_o = nc.dram_tensor("zout_o", [128, NSLOT], f32, kind="ExternalOutput")
    sq_o = nc.dram_tensor("sq_o", [128, NSLOT], f32, kind="ExternalOutput")
    ct_o = nc.dram_tensor("ct_o", [128, 3], f32, kind="ExternalOutput")
    ld_o = nc.dram_tensor("ld_o", [128, 3], f32, kind="ExternalOutput")

    with tile.TileContext(nc) as tc:
        with (
            tc.tile_pool(name="xtp", bufs=2) as xt_pool,
            tc.tile_pool(name="ftp", bufs=2) as ft_pool,
            tc.tile_pool(name="scr", bufs=2) as scr_pool,
            tc.tile_pool(name="slp", bufs=1) as sl_pool,
            tc.tile_pool(name="res", bufs=1) as res_pool,
            tc.tile_pool(name="psp", bufs=4, space="PSUM") as ps_pool,
        ):
            import contextlib
            with contextlib.ExitStack() as _rep:
                if reps > 1:  # timing-only: repeat the whole body on-device
                    _rep.enter_context(tc.For_i(0, reps, 1))
                _emit_body(nc, tc, xt_pool, ft_pool, scr_pool, sl_pool,
                           res_pool, ps_pool, xt_d, ft_d, xs_d, ts_d, g_d,
                           zout_o, sq_o, ct_o, ld_o, skip_act, skip_mm)
    if LEGALIZE:
        _legalize_sync_waits(nc)
    return nc


def _emit_body(nc, tc, xt_pool, ft_pool, scr_pool, sl_pool, res_pool,
               ps_pool, xt_d, ft_d, xs_d, ts_d, g_d, zout_o, sq_o, ct_o,
               ld_o, skip_act=False, skip_mm=False):
    f32 = mybir.dt.float32
    bf16 = mybir.dt.bfloat16
    if True:
        if True:
            zout_sb = res_pool.tile([128, NSLOT], f32, name="zout_sb")
            sq_sb = res_pool.tile([128, NSLOT], f32, name="sq_sb")
            ct_sb = res_pool.tile([128, 3], f32, name="ct_sb")
            ld_sb = res_pool.tile([128, 3], f32, name="ld_sb")
            beta_sb = res_pool.tile([128, 1], f32, name="beta_sb")
            nc.vector.memset(beta_sb, QBETA)

            for b in range(3):
                # --- stationary lhsT: all 16 x^T k-tiles in ONE 4MB DMA ---
                xt_big = xt_pool.tile([128, KT, B], bf16, name="xt_big",
                                      tag="xtb")
                nc.sync.dma_start(
                    out=xt_big,
                    in_=xt_d[b].ap().rearrange("(k p) i -> p k i", p=128))

                for jc in range(NJC):
                    # 16 ft k-tiles (1024-wide j chunk) in ONE 4MB DMA
                    ft_big = ft_pool.tile([128, KT, JCW], bf16, name="ft_big",
                                          tag="ftb")
                    nc.sync.dma_start(
                        out=ft_big,
                        in_=ft_d[b].ap().rearrange("(k p) j -> p k j", p=128)[
                            :, :, jc * JCW:(jc + 1) * JCW])
                    for it in range(NIT):
                        ps = ps_pool.tile([128, JCW], mybir.dt.float32,
                                          name="ps", tag="ps")
                        kt_eff = 1 if skip_mm else KT
                        for k in range(kt_eff):
                            lhsT = xt_big[:, k, it * 128:(it + 1) * 128]
                            for h in range(2):
                                nc.tensor.matmul(
                                    ps[:, h * 512:(h + 1) * 512],
                                    lhsT,
                                    ft_big[:, k, h * 512:(h + 1) * 512],
                                    start=(k == 0), stop=(k == kt_eff - 1))
                        idx = (b * NJC + jc) * NIT + it
                        if not skip_act:
                            e1 = scr_pool.tile([128, JCW], bf16, name="e1",
                                               tag="e1")
                            nc.scalar.activation(
                                e1, ps, mybir.ActivationFunctionType.Exp,
                                scale=1.0 / TEMP,
                                accum_out=zout_sb[:, idx:idx + 1])
                            sqs = scr_pool.tile([128, JCW], bf16, name="sqs",
                                                tag="sqs")
                            nc.scalar.activation(
                                sqs, ps, mybir.ActivationFunctionType.Square,
                                bias=beta_sb, scale=1.0,
                                accum_out=sq_sb[:, idx:idx + 1])

                # --- per-core row-slice work: MSE partials + target dots ---
                # (emitted after the matmul stream so the big DMAs go first)
                xs_t = sl_pool.tile([128, D], bf16, name="xs_t", tag="xs")
                nc.sync.dma_start(out=xs_t, in_=xs_d[b].ap())
                ts_t = sl_pool.tile([128, D], bf16, name="ts_t", tag="ts")
                nc.sync.dma_start(out=ts_t, in_=ts_d[b].ap())
                g_t = sl_pool.tile([128, D], bf16, name="g_t", tag="g")
                nc.sync.dma_start(out=g_t, in_=g_d[b].ap())

                diff_t = sl_pool.tile([128, D], bf16, name="diff_t", tag="diff")
                nc.vector.tensor_sub(diff_t, xs_t, ts_t)
                msescr = sl_pool.tile([128, D], bf16, name="msescr", tag="msescr")
                nc.scalar.activation(msescr, diff_t,
                                     mybir.ActivationFunctionType.Square,
                                     accum_out=ld_sb[:, b:b + 1])
                ctscr = sl_pool.tile([128, D], bf16, name="ctscr", tag="ctscr")
                nc.vector.scalar_tensor_tensor(
                    ctscr, xs_t, 0.0, g_t,
                    op0=mybir.AluOpType.add, op1=mybir.AluOpType.mult,
                    accum_out=ct_sb[:, b:b + 1])

            if not skip_act:
                nc.sync.dma_start(out=zout_o.ap(), in_=zout_sb)
                nc.sync.dma_start(out=sq_o.ap(), in_=sq_sb)
            nc.sync.dma_start(out=ct_o.ap(), in_=ct_sb)
            nc.sync.dma_start(out=ld_o.ap(), in_=ld_sb)


def _l2norm_rows(a):
    n = np.sqrt(np.sum(a.astype(np.float64) ** 2, axis=1, keepdims=True))
    return a / np.maximum(n, 1e-12)


def _prep_in_maps(students, teachers, banks, tgt):
    """Host-side shard prep: l2norm, transpose, bf16 cast, target-row gather.
    Returns (in_maps, xn, g_rows)."""
    xn = [_l2norm_rows(s) for s in students]            # float64 [B, D]
    tn = [_l2norm_rows(t) for t in teachers]
    xt_bf = [np.ascontiguousarray(x.T.astype(np.float32)).astype(BF16)
             for x in xn]                               # [D, B] bf16
    ft_bf = [np.ascontiguousarray(f.T).astype(BF16) for f in banks]  # [D, N]
    g_rows = [f[tgt] for f in banks]                    # [B, D] float32

    in_maps = []
    for c in range(NCORES):
        rs = slice(c * RSH, (c + 1) * RSH)
        m = {}
        for b in range(3):
            m[f"xt{b}"] = xt_bf[b]
            m[f"ft{b}"] = np.ascontiguousarray(
                ft_bf[b][:, c * JSH:(c + 1) * JSH])
            m[f"xs{b}"] = xn[b][rs].astype(np.float32).astype(BF16)
            m[f"tn{b}"] = tn[b][rs].astype(np.float32).astype(BF16)
            m[f"g{b}"] = g_rows[b][rs].astype(BF16)
        in_maps.append(m)
    return in_maps, xn, g_rows


def kernel(inputs, inputs_up, inputs_down, inputs_teacher, inputs_up_teacher,
           inputs_down_teacher, targets, epoch, features, features_up,
           features_down):
    global LAST_RESULTS
    students = [np.asarray(x, np.float32) for x in
                (inputs, inputs_up, inputs_down)]
    teachers = [np.asarray(x, np.float32) for x in
                (inputs_teacher, inputs_up_teacher, inputs_down_teacher)]
    banks = [np.asarray(x, np.float32) for x in
             (features, features_up, features_down)]
    tgt = np.asarray(targets).astype(np.int64)

    in_maps, xn, g_rows = _prep_in_maps(students, teachers, banks, tgt)

    if "nc" not in _NC_CACHE:
        _NC_CACHE["nc"] = _build_nc()
    nc = _NC_CACHE["nc"]

    res = run_bass_kernel_spmd(nc, in_maps, core_ids=list(range(NCORES)),
                               trace=TRACE, **TRACE_KWARGS)
    LAST_RESULTS = res

    # host combine: [128, 48] slot layout is (p, (b, jc, it))
    zout = np.zeros((3, NIT, 128), np.float64)
    sqacc = np.zeros((3, NIT, 128), np.float64)
    ct = np.zeros((3, B), np.float64)
    ld = np.zeros(3, np.float64)
    for c in range(NCORES):
        r = res.results[c]
        zo = r["zout_o"].astype(np.float64).reshape(128, 3, NJC, NIT)
        sq = r["sq_o"].astype(np.float64).reshape(128, 3, NJC, NIT)
        zout += zo.sum(axis=2).transpose(1, 2, 0)
        sqacc += sq.sum(axis=2).transpose(1, 2, 0)
        ct[:, c * RSH:(c + 1) * RSH] = r["ct_o"].astype(np.float64).T
        ld += r["ld_o"].astype(np.float64).sum(axis=0)
    zout = zout.reshape(3, B)    # row i = it*128 + p
    sqacc = sqacc.reshape(3, B)
    ld /= B

    zd = QA2 * sqacc + N * QC0   # sum_j exp(dist_ij), via quadratic surrogate

    loss = 0.0
    weights = [1.0 - LAMBDA2, LAMBDA2, LAMBDA2]
    for b in range(3):
        x2 = np.sum(xn[b] ** 2, axis=1)          # ~1, matches reference cdist
        f2t = np.sum(g_rows[b].astype(np.float64) ** 2, axis=1)
        ce_out = np.mean(np.log(zout[b]) - ct[b] / TEMP)
        d_t = np.sqrt(np.maximum(x2 + f2t - 2.0 * ct[b], 0.0))
        s_t = np.exp(d_t) / zd[b]
        ce_soft = np.log(float(N + 1)) - np.mean(s_t)
        loss += weights[b] * (ce_out + MU * ld[b] + ce_soft)

    return np.float32(loss)



# revision 2
# speedup vs baseline: 1.9094x; 1.9094x over previous
"""Trainium2 Bass kernel for nn_ClusterMemory (scatter_memory).

Strategy
--------
Column-shard ("tensor parallel") the three memory banks along num_samples:
core c owns bank columns [c*2048, (c+1)*2048).  Every core receives the full
(l2-normalized, transposed) student batch, quantized to fp8e4 with a 64x
per-side scale, and computes its [1024, 2048] block of the three similarity
matrices C_b = x_b @ F_b^T on the PE in fp8 DoubleRow mode (2 MACs per cell
per cycle -> 2x bf16 throughput).  The only device reduction is the ACT
engine's Exp-with-accumulate over each PSUM tile, producing per-row partial
sums of exp(C/T) -> CE(out)'s log-sum-exp.

Loss decomposition (everything else is O(B*D) and done on host in float64):

  CE(out_b)    = mean_i [ log(sum_j exp(C/T)) - C[i,t_i]/T ]
                 -> device: row-sums of exp(C/T) via ACT Exp+accum (the fp8
                    matmul feeds it; psum carries 4096*c, Exp scale folds
                    the 1/4096 and 1/T).
                 -> C[i,t_i] = <x_i, f_{t_i}> on host (exact, O(B*D)).
  MSE ld_b     = sum_d mean_i (x - t)^2  -> host (exact, O(B*D)).
  CE(soft_b)   = mean_i [ log(sum_j exp(s_ij)) - s[i,t_i] ],
                 s = softmax_j(dist).  dist in [0,2] => s_ij ~ 6e-5, so
                 sum_j exp(s_ij) = N + 1 + O(1e-4) and the whole term is
                 log(N+1) - mean_i s_t to ~1e-9.  s_t = exp(d_t)/Zd_i where
                 Zd_i = sum_j exp(dist_ij) ~ N*e^sqrt(2) with only ~1e-4
                 row-to-row variation; since s_t itself only contributes
                 ~1e-4 to the loss, Zd is evaluated by the Gaussian-weighted
                 linear fit  exp(sqrt(2-2c)) ~ ZA + ZB*c  (c ~ N(0,1/D)),
                 giving Zd_i = N*ZA + ZB * <x_i, sum_j f_j> -- one host
                 matvec, no device work.  Residual O(c^2) terms concentrate
                 (sum_j c_ij^2 ~ N/D +- 1e-3) so the surrogate tracks true
                 Zd to ~1e-4 relative; loss impact < 1e-8.

fp8 error budget: per-side quantization err ~3.6% rms on N(0,1) entries
-> cosine err ~1.1e-3 -> logit err ~0.022 -> log-sum-exp bias +2.5e-4
absolute per CE term (loss ~32) -> ~1e-5 relative.  C[i,t_i] is exact
(host).  Well inside the 2e-3 gate.
"""

import numpy as np
import ml_dtypes

import bass_rust
import concourse.bass as bass
import concourse.tile as tile
from concourse import mybir
from concourse.bass_utils import run_bass_kernel_spmd

B, D, N = 1024, 2048, 16384
TEMP, LAMBDA2, MU = 0.05, 0.5, 1.0
NCORES = 8
JSH = N // NCORES          # 2048 bank columns per core
KT = D // 128              # 16 contraction tiles of 128
KH = KT // 2               # 8 per DMA half
NIT = B // 128             # 8 row tiles
NSLOT = 3 * NIT            # 24 accumulation slots (b, it)

F8 = ml_dtypes.float8_e4m3   # TRN fp8_exp4: bias 7, max normal 240
SCALE = 64.0                 # per-side fp8 scale; psum carries SCALE^2 * c

# Gaussian-weighted linear fit of f(c) = exp(sqrt(2 - 2c)) for c ~ N(0, 1/D):
# Zd_i = sum_j f(c_ij) ~ N*ZA + ZB * sum_j c_ij.
_sig = 1.0 / np.sqrt(D)
_c = np.linspace(-8.0 * _sig, 8.0 * _sig, 8001)
_w = np.exp(-0.5 * (_c / _sig) ** 2)
_f = np.exp(np.sqrt(2.0 - 2.0 * _c))
_m00, _m01, _m11 = _w.sum(), (_w * _c).sum(), (_w * _c * _c).sum()
_r0, _r1 = (_w * _f).sum(), (_w * _c * _f).sum()
ZA, ZB = np.linalg.solve([[_m00, _m01], [_m01, _m11]], [_r0, _r1])

_NC_CACHE = {}
TRACE = False
TRACE_KWARGS = {}
LAST_RESULTS = None
LEGALIZE = True  # hardware needs at most one sync wait per instruction


def _legalize_sync_waits(nc):
    """The walrus build in this container encodes at most one sync wait per
    instruction; hoist extra waits into standalone EventSemaphore sequencer
    instructions on the same engine immediately before the instruction
    (identical semantics: the sequencer blocks before issuing)."""
    f = nc.m.functions[0]
    for blk in f.blocks:
        out = []
        for ins in blk.instructions:
            si = ins.sync_info
            if si is not None:
                waits = list(si.on_wait)
                ups = list(si.on_update or [])
                assert len(ups) <= 1, ins.concise()
                if len(waits) > 1:
                    for w in waits[:-1]:
                        ev = mybir.InstEventSemaphore(
                            name=f"lgw-{nc.next_id()}", ins=[], outs=[])
                        ev.engine = ins.engine
                        ev.sync_info = bass_rust.SyncInfo(on_wait=[w],
                                                          on_update=[])
                        out.append(ev)
                    ins.sync_info = bass_rust.SyncInfo(on_wait=[waits[-1]],
                                                      on_update=ups)
            out.append(ins)
        blk.instructions = out


def _build_nc():
    f32 = mybir.dt.float32
    bf16 = mybir.dt.bfloat16
    f8 = mybir.dt.float8e4
    nc = bass.Bass("TRN2", target_bir_lowering=False, debug=False,
                   num_devices=NCORES)

    xt_d = [nc.dram_tensor(f"xt{b}", [D, B], f8, kind="ExternalInput")
            for b in range(3)]
    ft_d = [nc.dram_tensor(f"ft{b}", [D, JSH], f8, kind="ExternalInput")
            for b in range(3)]
    zout_o = nc.dram_tensor("zout_o", [128, NSLOT], f32, kind="ExternalOutput")

    with tile.TileContext(nc) as tc:
        with (
            tc.tile_pool(name="xtp", bufs=2) as xt_pool,
            tc.tile_pool(name="ftp", bufs=2) as ft_pool,
            tc.tile_pool(name="scr", bufs=2) as scr_pool,
            tc.tile_pool(name="res", bufs=1) as res_pool,
            tc.tile_pool(name="psp", bufs=2, space="PSUM") as ps_pool,
        ):
            zout_sb = res_pool.tile([128, NSLOT], f32, name="zout_sb")

            for b in range(3):
                # k-halved DMAs so the first matmuls start after ~3MB
                xt_lo = xt_pool.tile([128, KH, B], f8, name="xt_lo", tag="xlo")
                nc.sync.dma_start(
                    out=xt_lo,
                    in_=xt_d[b].ap().rearrange("(k p) i -> p k i",
                                               p=128)[:, 0:KH, :])
                ft_lo = ft_pool.tile([128, KH, JSH], f8, name="ft_lo",
                                     tag="flo")
                nc.sync.dma_start(
                    out=ft_lo,
                    in_=ft_d[b].ap().rearrange("(k p) j -> p k j",
                                               p=128)[:, 0:KH, :])
                xt_hi = xt_pool.tile([128, KH, B], f8, name="xt_hi", tag="xhi")
                nc.sync.dma_start(
                    out=xt_hi,
                    in_=xt_d[b].ap().rearrange("(k p) i -> p k i",
                                               p=128)[:, KH:KT, :])
                ft_hi = ft_pool.tile([128, KH, JSH], f8, name="ft_hi",
                                     tag="fhi")
                nc.sync.dma_start(
                    out=ft_hi,
                    in_=ft_d[b].ap().rearrange("(k p) j -> p k j",
                                               p=128)[:, KH:KT, :])

                for it in range(NIT):
                    ps = ps_pool.tile([128, JSH], mybir.dt.float32,
                                      name="ps", tag="ps")
                    for k2 in range(KH):  # 8 DoubleRow steps of 256-deep K
                        if k2 < KH // 2:
                            xsrc, fsrc, ko = xt_lo, ft_lo, k2
                        else:
                            xsrc, fsrc, ko = xt_hi, ft_hi, k2 - KH // 2
                        lhsT = xsrc[:, 2 * ko:2 * ko + 2,
                                    it * 128:(it + 1) * 128]
                        for h in range(JSH // 512):
                            nc.tensor.matmul(
                                ps[:, h * 512:(h + 1) * 512],
                                lhsT,
                                fsrc[:, 2 * ko:2 * ko + 2,
                                     h * 512:(h + 1) * 512],
                                start=(k2 == 0), stop=(k2 == KH - 1),
                                perf_mode=mybir.MatmulPerfMode.DoubleRow)
                    idx = b * NIT + it
                    e1 = scr_pool.tile([128, JSH], bf16, name="e1", tag="e1")
                    nc.scalar.activation(
                        e1, ps, mybir.ActivationFunctionType.Exp,
                        scale=1.0 / (SCALE * SCALE * TEMP),
                        accum_out=zout_sb[:, idx:idx + 1])

            nc.sync.dma_start(out=zout_o.ap(), in_=zout_sb)
    if LEGALIZE:
        _legalize_sync_waits(nc)
    return nc


def _l2norm_rows(a):
    n = np.sqrt(np.sum(a.astype(np.float64) ** 2, axis=1, keepdims=True))
    return a / np.maximum(n, 1e-12)


def kernel(inputs, inputs_up, inputs_down, inputs_teacher, inputs_up_teacher,
           inputs_down_teacher, targets, epoch, features, features_up,
           features_down):
    global LAST_RESULTS
    students = [np.asarray(x, np.float32) for x in
                (inputs, inputs_up, inputs_down)]
    teachers = [np.asarray(x, np.float32) for x in
                (inputs_teacher, inputs_up_teacher, inputs_down_teacher)]
    banks = [np.asarray(x, np.float32) for x in
             (features, features_up, features_down)]
    tgt = np.asarray(targets).astype(np.int64)

    xn = [_l2norm_rows(s) for s in students]            # float64 [B, D]
    tn = [_l2norm_rows(t) for t in teachers]
    xt_f8 = [np.ascontiguousarray(x.T * SCALE).astype(np.float32).astype(F8)
             for x in xn]                               # [D, B] fp8
    ft_f8 = [(f.T.astype(np.float64) * SCALE).astype(np.float32).astype(F8)
             for f in banks]                            # [D, N] fp8
    ft_f8 = [np.ascontiguousarray(f) for f in ft_f8]

    in_maps = []
    for c in range(NCORES):
        m = {}
        for b in range(3):
            m[f"xt{b}"] = xt_f8[b]
            m[f"ft{b}"] = np.ascontiguousarray(
                ft_f8[b][:, c * JSH:(c + 1) * JSH])
        in_maps.append(m)

    if "nc" not in _NC_CACHE:
        _NC_CACHE["nc"] = _build_nc()
    nc = _NC_CACHE["nc"]

    res = run_bass_kernel_spmd(nc, in_maps, core_ids=list(range(NCORES)),
                               trace=TRACE, **TRACE_KWARGS)
    LAST_RESULTS = res

    # host combine: [128, 24] slot layout is (p, (b, it)); row i = it*128 + p
    zout = np.zeros((3, NIT, 128), np.float64)
    for c in range(NCORES):
        zo = res.results[c]["zout_o"].astype(np.float64).reshape(128, 3, NIT)
        zout += zo.transpose(1, 2, 0)
    zout = zout.reshape(3, B)

    loss = 0.0
    weights = [1.0 - LAMBDA2, LAMBDA2, LAMBDA2]
    for b in range(3):
        g = banks[b][tgt].astype(np.float64)             # [B, D] target rows
        ct = np.einsum("ij,ij->i", xn[b], g)             # C[i, t_i], exact
        ld = np.sum(np.mean((xn[b] - tn[b]) ** 2, axis=0))
        x2 = np.sum(xn[b] ** 2, axis=1)                  # ~1, matches cdist
        f2t = np.sum(g ** 2, axis=1)
        ce_out = np.mean(np.log(zout[b])) - np.mean(ct) / TEMP
        d_t = np.sqrt(np.maximum(x2 + f2t - 2.0 * ct, 0.0))
        s_col = xn[b] @ banks[b].astype(np.float64).sum(axis=0)  # sum_j c_ij
        zd = N * ZA + ZB * s_col
        ce_soft = np.log(float(N + 1)) - np.mean(np.exp(d_t) / zd)
        loss += weights[b] * (ce_out + MU * ld + ce_soft)

    return np.float32(loss)


# revision 4
# speedup vs baseline: 1.9474x; 1.0199x over previous
"""Trainium2 Bass kernel for nn_ClusterMemory (scatter_memory).

Strategy
--------
Column-shard ("tensor parallel") the three memory banks along num_samples:
core c owns bank columns [c*2048, (c+1)*2048).  Every core receives the full
(l2-normalized, transposed) student batch, quantized to fp8e4 with a 64x
per-side scale, and computes its [1024, 2048] block of the three similarity
matrices C_b = x_b @ F_b^T on the PE in fp8 DoubleRow mode (2 MACs per cell
per cycle -> 2x bf16 throughput; 768 N=512 matmuls/core = the 164us fp8
roofline).  The only device reduction is the ACT engine's Exp-with-
accumulate over each PSUM bank, producing per-row partial sums of exp(C/T)
-> CE(out)'s log-sum-exp.

Inputs are host-swizzled to partition-major chunks (xt per 128-row tile,
ft per 512-wide j strip) so DMA lines are 2KB+ (full rate) and the first
matmul's gating set is only ft-strip0 + xt-it0 ~ 1.25MB.  xt chunks go out
on the Vector engine's DMA queue, ft strips on Sync's, so the transfers
overlap.  A burst of warm-up matmuls on a zeroed tile runs during the
DMA wait to hold the PE's HAM clock-gate at full rate.

Loss decomposition (everything else is O(B*D) and done on host in float64):

  CE(out_b)    = mean_i [ log(sum_j exp(C/T)) - C[i,t_i]/T ]
                 -> device: row-sums of exp(C/T) via ACT Exp+accum (psum
                    carries 4096*c; Exp scale folds the 1/4096 and 1/T).
                 -> C[i,t_i] = <x_i, f_{t_i}> on host (exact, O(B*D)).
  MSE ld_b     = sum_d mean_i (x - t)^2  -> host (exact, O(B*D)).
  CE(soft_b)   = mean_i [ log(sum_j exp(s_ij)) - s[i,t_i] ],
                 s = softmax_j(dist).  dist in [0,2] => s_ij ~ 6e-5, so
                 sum_j exp(s_ij) = N + 1 + O(1e-4) and the whole term is
                 log(N+1) - mean_i s_t to ~1e-9.  s_t = exp(d_t)/Zd_i where
                 Zd_i = sum_j exp(dist_ij) ~ N*e^sqrt(2) with only ~1e-4
                 row-to-row variation; since s_t itself only contributes
                 ~1e-4 to the loss, Zd is evaluated by the Gaussian-weighted
                 linear fit  exp(sqrt(2-2c)) ~ ZA + ZB*c  (c ~ N(0,1/D)),
                 giving Zd_i = N*ZA + ZB * <x_i, sum_j f_j> -- one host
                 matvec (validated: ~8e-7 rel err vs exact on real data).

fp8 error budget: per-side quantization err ~3.6% rms on N(0,1) entries
-> cosine err ~1.1e-3 -> logit err ~0.022 -> log-sum-exp bias +2.5e-4
absolute per CE term (loss ~32) -> ~1e-5 relative.  C[i,t_i] is exact
(host).  Well inside the 2e-3 gate.
"""

import numpy as np
import ml_dtypes

import bass_rust
import concourse.bass as bass
import concourse.tile as tile
from concourse import mybir
from concourse.bass_utils import run_bass_kernel_spmd

B, D, N = 1024, 2048, 16384
TEMP, LAMBDA2, MU = 0.05, 0.5, 1.0
NCORES = 8
JSH = N // NCORES          # 2048 bank columns per core
KT = D // 128              # 16 contraction tiles of 128
NIT = B // 128             # 8 row tiles
NJC = 4                    # j strips per core
JCW = JSH // NJC           # 512 = one PSUM bank
NSLOT = 3 * NJC * NIT      # 96 accumulation slots ((b, jc), it)
NWARM = 24                 # HAM warm-up matmuls during the prologue DMA

F8 = ml_dtypes.float8_e4m3   # TRN fp8_exp4: bias 7, max normal 240
SCALE = 64.0                 # per-side fp8 scale; psum carries SCALE^2 * c

# Gaussian-weighted linear fit of f(c) = exp(sqrt(2 - 2c)) for c ~ N(0, 1/D):
# Zd_i = sum_j f(c_ij) ~ N*ZA + ZB * sum_j c_ij.
_sig = 1.0 / np.sqrt(D)
_c = np.linspace(-8.0 * _sig, 8.0 * _sig, 8001)
_w = np.exp(-0.5 * (_c / _sig) ** 2)
_f = np.exp(np.sqrt(2.0 - 2.0 * _c))
_m00, _m01, _m11 = _w.sum(), (_w * _c).sum(), (_w * _c * _c).sum()
_r0, _r1 = (_w * _f).sum(), (_w * _c * _f).sum()
ZA, ZB = np.linalg.solve([[_m00, _m01], [_m01, _m11]], [_r0, _r1])

_NC_CACHE = {}
TRACE = False
TRACE_KWARGS = {}
LAST_RESULTS = None
LEGALIZE = True  # hardware needs at most one sync wait per instruction


def _legalize_sync_waits(nc):
    """The walrus build in this container encodes at most one sync wait per
    instruction; hoist extra waits into standalone EventSemaphore sequencer
    instructions on the same engine immediately before the instruction
    (identical semantics: the sequencer blocks before issuing)."""
    f = nc.m.functions[0]
    for blk in f.blocks:
        out = []
        for ins in blk.instructions:
            si = ins.sync_info
            if si is not None:
                waits = list(si.on_wait)
                ups = list(si.on_update or [])
                assert len(ups) <= 1, ins.concise()
                if len(waits) > 1:
                    for w in waits[:-1]:
                        ev = mybir.InstEventSemaphore(
                            name=f"lgw-{nc.next_id()}", ins=[], outs=[])
                        ev.engine = ins.engine
                        ev.sync_info = bass_rust.SyncInfo(on_wait=[w],
                                                          on_update=[])
                        out.append(ev)
                    ins.sync_info = bass_rust.SyncInfo(on_wait=[waits[-1]],
                                                      on_update=ups)
            out.append(ins)
        blk.instructions = out


def _build_nc():
    f32 = mybir.dt.float32
    bf16 = mybir.dt.bfloat16
    f8 = mybir.dt.float8e4
    DR = mybir.MatmulPerfMode.DoubleRow
    nc = bass.Bass("TRN2", target_bir_lowering=False, debug=False,
                   num_devices=NCORES)

    # host-swizzled layouts: xt rows (it*128+p) hold [KT,128] i-chunks;
    # ft rows (jc*128+p) hold [KT,512] j-strips.  2KB+ contiguous per line.
    xt_d = [nc.dram_tensor(f"xt{b}", [NIT * 128, KT * 128], f8,
                           kind="ExternalInput") for b in range(3)]
    ft_d = [nc.dram_tensor(f"ft{b}", [NJC * 128, KT * JCW], f8,
                           kind="ExternalInput") for b in range(3)]
    zout_o = nc.dram_tensor("zout_o", [128, NSLOT], f32, kind="ExternalOutput")

    with tile.TileContext(nc) as tc:
        with (
            tc.tile_pool(name="xtp", bufs=2) as xt_pool,
            tc.tile_pool(name="ftp", bufs=2) as ft_pool,
            tc.tile_pool(name="scr", bufs=3) as scr_pool,
            tc.tile_pool(name="res", bufs=1) as res_pool,
            tc.tile_pool(name="psp", bufs=7, space="PSUM") as ps_pool,
            tc.tile_pool(name="wps", bufs=1, space="PSUM") as wps_pool,
        ):
            zout_sb = res_pool.tile([128, NSLOT], f32, name="zout_sb")

            # HAM warm-up: keep the PE busy during the prologue DMA so the
            # clock-gate is at 8/8 when the real matmuls start.
            wsrc = res_pool.tile([128, 2, 512], f8, name="wsrc")
            nc.vector.memset(wsrc, 0)
            wps = wps_pool.tile([128, 512], f32, name="wps")
            for _ in range(NWARM):
                nc.tensor.matmul(wps, wsrc[:, :, 0:128], wsrc,
                                 start=True, stop=True, perf_mode=DR)

            for b in range(3):
                ft_sb = []
                for jc in range(NJC):
                    t = ft_pool.tile([128, KT, JCW], f8, name=f"ft{jc}",
                                     tag=f"ft{jc}")
                    # ft strips on the Sync engine's DMA queue
                    nc.sync.dma_start(
                        out=t,
                        in_=ft_d[b].ap()[jc * 128:(jc + 1) * 128, :]
                        .rearrange("p (k j) -> p k j", k=KT))
                    ft_sb.append(t)
                xt_sb = xt_pool.tile([128, NIT, KT, 128], f8, name="xt_sb",
                                     tag="xt")
                for it in range(NIT):
                    # xt row-tile chunks on the GpSimd engine's DMA queue
                    nc.gpsimd.dma_start(
                        out=xt_sb[:, it, :, :],
                        in_=xt_d[b].ap()[it * 128:(it + 1) * 128, :]
                        .rearrange("p (k i) -> p k i", k=KT))

                for jc in range(NJC):
                    for it in range(NIT):
                        ps = ps_pool.tile([128, JCW], f32, name="ps",
                                          tag="ps")
                        for k2 in range(KT // 2):
                            lhsT = xt_sb[:, it, 2 * k2:2 * k2 + 2, :]
                            nc.tensor.matmul(
                                ps, lhsT,
                                ft_sb[jc][:, 2 * k2:2 * k2 + 2, :],
                                start=(k2 == 0), stop=(k2 == KT // 2 - 1),
                                perf_mode=DR)
                        idx = (b * NJC + jc) * NIT + it
                        e1 = scr_pool.tile([128, JCW], bf16, name="e1",
                                           tag="e1")
                        nc.scalar.activation(
                            e1, ps, mybir.ActivationFunctionType.Exp,
                            scale=1.0 / (SCALE * SCALE * TEMP),
                            accum_out=zout_sb[:, idx:idx + 1])

                # per-bank output DMA so the tail only waits on bank 2
                lo, hi = b * NJC * NIT, (b + 1) * NJC * NIT
                nc.sync.dma_start(out=zout_o.ap()[:, lo:hi],
                                  in_=zout_sb[:, lo:hi])
    if LEGALIZE:
        _legalize_sync_waits(nc)
    return nc


def _l2norm_rows(a):
    n = np.sqrt(np.sum(a.astype(np.float64) ** 2, axis=1, keepdims=True))
    return a / np.maximum(n, 1e-12)


def kernel(inputs, inputs_up, inputs_down, inputs_teacher, inputs_up_teacher,
           inputs_down_teacher, targets, epoch, features, features_up,
           features_down):
    global LAST_RESULTS
    students = [np.asarray(x, np.float32) for x in
                (inputs, inputs_up, inputs_down)]
    teachers = [np.asarray(x, np.float32) for x in
                (inputs_teacher, inputs_up_teacher, inputs_down_teacher)]
    banks = [np.asarray(x, np.float32) for x in
             (features, features_up, features_down)]
    tgt = np.asarray(targets).astype(np.int64)

    xn = [_l2norm_rows(s) for s in students]            # float64 [B, D]
    tn = [_l2norm_rows(t) for t in teachers]

    # device layouts: xt [(it p), (k i)], ft [(jc p), (k j)] per core
    xt_f8 = []
    for x in xn:
        a = (x.T * SCALE).astype(np.float32).astype(F8)        # [D, B]
        a = a.reshape(KT, 128, NIT, 128).transpose(2, 1, 0, 3)
        xt_f8.append(np.ascontiguousarray(a.reshape(NIT * 128, KT * 128)))
    ft_f8_full = [(f.T.astype(np.float32) * SCALE).astype(F8)  # [D, N]
                  for f in banks]

    in_maps = []
    for c in range(NCORES):
        m = {}
        for b in range(3):
            m[f"xt{b}"] = xt_f8[b]
            fc = ft_f8_full[b][:, c * JSH:(c + 1) * JSH]
            fc = fc.reshape(KT, 128, NJC, JCW).transpose(2, 1, 0, 3)
            m[f"ft{b}"] = np.ascontiguousarray(
                fc.reshape(NJC * 128, KT * JCW))
        in_maps.append(m)

    if "nc" not in _NC_CACHE:
        _NC_CACHE["nc"] = _build_nc()
    nc = _NC_CACHE["nc"]

    res = run_bass_kernel_spmd(nc, in_maps, core_ids=list(range(NCORES)),
                               trace=TRACE, **TRACE_KWARGS)
    LAST_RESULTS = res

    # host combine: [128, 96] slots are (p, (b, jc, it)); row i = it*128 + p
    zout = np.zeros((3, NIT, 128), np.float64)
    for c in range(NCORES):
        zo = res.results[c]["zout_o"].astype(np.float64)
        zout += zo.reshape(128, 3, NJC, NIT).sum(axis=2).transpose(1, 2, 0)
    zout = zout.reshape(3, B)

    loss = 0.0
    weights = [1.0 - LAMBDA2, LAMBDA2, LAMBDA2]
    for b in range(3):
        g = banks[b][tgt].astype(np.float64)             # [B, D] target rows
        ct = np.einsum("ij,ij->i", xn[b], g)             # C[i, t_i], exact
        ld = np.sum(np.mean((xn[b] - tn[b]) ** 2, axis=0))
        x2 = np.sum(xn[b] ** 2, axis=1)                  # ~1, matches cdist
        f2t = np.sum(g ** 2, axis=1)
        ce_out = np.mean(np.log(zout[b])) - np.mean(ct) / TEMP
        d_t = np.sqrt(np.maximum(x2 + f2t - 2.0 * ct, 0.0))
        s_col = xn[b] @ banks[b].astype(np.float64).sum(axis=0)  # sum_j c_ij
        zd = N * ZA + ZB * s_col
        ce_soft = np.log(float(N + 1)) - np.mean(np.exp(d_t) / zd)
        loss += weights[b] * (ce_out + MU * ld + ce_soft)

    return np.float32(loss)
